# revision 26
# baseline (speedup 1.0000x reference)
"""CAMSA multi-mask attention kernel for one TRN2 chip (8 NeuronCores).

Problem: B=4, S=2048, D=1024, M=4 stride masks.
  Q = x@Wq + bq ; K = x@Wk + bk ; V = x@Wv + bv     (biases are zero-fill)
  scores = Q K^T / sqrt(D)                           [B,S,S]
  weights_m = softmax(where(mask_m==0, -1e9, scores))
  out = (mean_m weights_m) @ V @ Wo + bo

Algebra: with s = scores/sqrt(D) (~N(0,1), no row-max needed) and the
host-precomputed mask bias mb_m = 30*(mask_m-1) in {-30, 0}:
  U_m = exp(s + mb_m) = mask_m * exp(s)   (exp(-30) ~ 1e-13 underflows)
  den_m[q] = sum_k U_m[q,k]  (ACT accumulator, free with the exp)
  inv_m = 1/(M*den_m);  Wsum = sum_m inv_m*U_m;  out = Wsum @ V @ Wo

Sharding: core c = (batch b=c//2, query-half h=c%2): 1024 query rows.
K/V are each computed for the OWN half of key rows only (from xTq) and
exchanged within the batch pair by an HBM AllGather over replica groups
[[0,1],[2,3],[4,5],[6,7]] - no duplicated projection work.

All inputs arrive fp16 from the host (x pre-transposed); output fp16.

Device pipeline per core (all matmuls fp16, contraction on partitions):
  A: DMA loads on gpsimd ring: xTq | Wk | Wq | Wv
  B: Kh proj -> [K AllGather] ; Q proj ; Vh proj -> [V AllGather]
     (gathers overlap the next projection on the tensor engine)
  C: per q-tile: scores (PSUM) -> ACT copy*1/sqrt(D) -> s16; per mask:
     DVE tensor_tensor add (2x) sm=s16+mb_m, ACT exp with accum_out
     (den_m free); reciprocal; Wsum via DVE tensor_scalar (4x) + adds.
  E: Wsum -> WsumT via one xbar DMA-transpose per q-tile
  F: out_preT[d,q] = V-tile^T . WsumT
  G: final[q,dout] = out_preT-tile^T . Wo -> DRAM (fp16)
"""

import numpy as np

B, S, D, M = 4, 2048, 1024, 4
SQ = S // 2          # query rows per core
PART = 128
N_CORES = 8

_CACHE = {}


def build(nc_factory=None, S=S, D=D, SQ=SQ, M=M, use_deps=True):
    from concourse import bass, mybir, bacc, tile
    from concourse.tile import add_dep_helper

    fp32 = mybir.dt.float32
    fp16 = mybir.dt.float16
    AF = mybir.ActivationFunctionType
    ALU = mybir.AluOpType

    P = PART
    DCH = D // P         # d-chunks
    KCH = S // P         # key-row chunks
    HKCH = KCH // 2      # key-row chunks per half
    QTILES = SQ // P     # q-tiles per core
    NB = min(512, S, SQ, D)
    GROUPS = [[0, 1], [2, 3], [4, 5], [6, 7]]

    if nc_factory is None:
        nc = bacc.Bacc("TRN2", target_bir_lowering=False, debug=False,
                       num_devices=N_CORES)
    else:
        nc = nc_factory()

    xT_d = nc.dram_tensor("xT", [D, S], fp16, kind="ExternalInput")
    xTq_d = nc.dram_tensor("xTq", [D, SQ], fp16, kind="ExternalInput")
    mk_d = nc.dram_tensor("mb", [M, SQ, S], fp16, kind="ExternalInput")
    wq_d = nc.dram_tensor("Wq", [D, D], fp16, kind="ExternalInput")
    wk_d = nc.dram_tensor("Wk", [D, D], fp16, kind="ExternalInput")
    wv_d = nc.dram_tensor("Wv", [D, D], fp16, kind="ExternalInput")
    wo_d = nc.dram_tensor("Wo", [D, D], fp16, kind="ExternalInput")
    out_d = nc.dram_tensor("out", [SQ, D], fp16, kind="ExternalOutput")

    with tile.TileContext(nc) as tc:
        with tc.tile_pool(name="persist", bufs=1) as pp, \
             tc.tile_pool(name="psum", bufs=8, space="PSUM") as psp, \
             tc.tile_pool(name="dram", bufs=1, space="DRAM") as dr:

            QT = pp.tile([P, DCH * SQ], fp16)    # [p, j*SQ+q] = Q[q, j*128+p]
            KT = pp.tile([P, DCH * S], fp16, tag="KT")  # [p,j*S+k] = K[k,j*128+p]
            V = pp.tile([P, KCH * D], fp16)      # [p, i*D+d]  = V[i*128+p, d]

            def wload(dst, src_d):
                return nc.gpsimd.dma_start(
                    dst[:].rearrange("p (c d) -> p c d", c=DCH),
                    src_d.ap().rearrange("(c p) d -> p c d", p=P))

            def proj(dst, w_sb, src_sb, ncols):
                # dst[p, j*ncols+r] = sum_dx W[dx, j*128+p] * src[dx, r]
                for j in range(DCH):
                    for qb in range(ncols // NB):
                        ps = psp.tile([P, NB], fp32, tag="ps", name="ps")
                        for c in range(DCH):
                            nc.tensor.matmul(
                                ps[:],
                                w_sb[:, c * D + j * P: c * D + (j + 1) * P],
                                src_sb[:, c * ncols + qb * NB: c * ncols + (qb + 1) * NB],
                                start=(c == 0), stop=(c == DCH - 1))
                        nc.vector.tensor_copy(
                            dst[:, j * ncols + qb * NB: j * ncols + (qb + 1) * NB],
                            ps[:])

            # ---- phase A/B: load + half projections + pair gathers ------
            with tc.tile_pool(name="stage", bufs=1) as sw:
                xTq = sw.tile([P, DCH * SQ], fp16, name="xTq")
                Wq = sw.tile([P, DCH * D], fp16, name="Wq")
                Wk = sw.tile([P, DCH * D], fp16, name="Wk")
                Wv = sw.tile([P, DCH * D], fp16, name="Wv")
                xT = sw.tile([P, DCH * S], fp16, name="xT")
                # xTq on the sync ring, parallel with the gp load chain
                nc.sync.dma_start(
                    xTq[:].rearrange("p (c r) -> p c r", c=DCH),
                    xTq_d.ap().rearrange("(c p) r -> p c r", p=P))
                d_wq = wload(Wq, wq_d)
                d_xt = nc.gpsimd.dma_start(
                    xT[:].rearrange("p (c r) -> p c r", c=DCH),
                    xT_d.ap().rearrange("(c p) r -> p c r", p=P))
                d_wk = wload(Wk, wk_d)
                d_wv = wload(Wv, wv_d)
                if use_deps:
                    add_dep_helper(d_xt.ins, d_wq.ins, sync=False, reason="dma order")
                    add_dep_helper(d_wk.ins, d_xt.ins, sync=False, reason="dma order")
                    add_dep_helper(d_wv.ins, d_wk.ins, sync=False, reason="dma order")

                proj(QT, Wq, xTq, SQ)
                proj(KT, Wk, xT, S)

                # V projection (full key range): lhsT = xT chunk, rhs = Wv
                for il in range(KCH):
                    for db in range(D // NB):
                        ps = psp.tile([P, NB], fp32, tag="ps", name="ps")
                        for c in range(DCH):
                            nc.tensor.matmul(
                                ps[:],
                                xT[:, c * S + il * P: c * S + (il + 1) * P],
                                Wv[:, c * D + db * NB: c * D + (db + 1) * NB],
                                start=(c == 0), stop=(c == DCH - 1))
                        nc.vector.tensor_copy(
                            V[:, il * D + db * NB: il * D + (db + 1) * NB],
                            ps[:])

            # ---- work pools for phases C/E/F/G --------------------------
            wk_ctx = tc.tile_pool(name="work", bufs=2)
            wkp = wk_ctx.__enter__()
            WT = wkp.tile([P, KCH * SQ], fp16, name="WT", tag="WT", bufs=1)
            #    [p, i*SQ+q] = Wsum[q, i*128+p]
            OT = wkp.tile([P, DCH * SQ], fp16, name="OT", tag="OT", bufs=1)
            #    [p, j*SQ+q] = out_pre[q, j*128+p]
            # mask-bias tiles [p, m*S+k] = 30*(mask[m, t*128+p, k]-1) stream
            # on the sync ring, two tiles ahead of consumption, so no
            # collective ever head-blocks them.
            mts = [None] * QTILES

            def issue_mt(t):
                mts[t] = wkp.tile([P, M * S], fp16, tag="mt", name="mt",
                                  bufs=3)
                nc.sync.dma_start(
                    mts[t][:].rearrange("p (m k) -> p m k", m=M),
                    mk_d.ap()[:, t * P:(t + 1) * P, :].transpose([1, 0, 2]))

            issue_mt(0)
            issue_mt(1)

            # ---- phase C/E: scores -> U_m -> Wsum -> WsumT --------------
            inv_scale = 1.0 / float(np.sqrt(np.float32(D)))
            for t in range(QTILES):
                if t + 2 < QTILES:
                    issue_mt(t + 2)
                mt = mts[t]

                s16 = wkp.tile([P, S], fp16, tag="s16", name="s16")
                for kb in range(S // NB):
                    ps = psp.tile([P, NB], fp32, tag="ps", name="ps")
                    for c in range(DCH):
                        nc.tensor.matmul(
                            ps[:],
                            QT[:, c * SQ + t * P: c * SQ + (t + 1) * P],
                            KT[:, c * S + kb * NB: c * S + (kb + 1) * NB],
                            start=(c == 0), stop=(c == DCH - 1))
                    nc.scalar.activation(
                        s16[:, kb * NB:(kb + 1) * NB], ps[:],
                        AF.Copy, scale=inv_scale)

                if t == QTILES - 1:
                    # Wo reuses KT's slot (KT dead after last scores).
                    Wo = pp.tile([P, DCH * D], fp16, name="Wo", tag="KT")
                    wload(Wo, wo_d)

                den = wkp.tile([P, M], fp32, tag="den", name="den")
                # sm_m = s + mb_m on DVE (2x); U_m = exp(sm_m) on ACT with
                # the row-sum accumulated for free.
                for m in range(M):
                    um = mt[:, m * S:(m + 1) * S]
                    nc.vector.tensor_tensor(um, um, s16[:], op=ALU.add)
                    nc.scalar.activation(um, um, AF.Exp,
                                         accum_out=den[:, m:m + 1])
                inv = wkp.tile([P, M], fp32, tag="inv", name="inv")
                nc.vector.reciprocal(inv[:], den[:])
                nc.vector.tensor_scalar_mul(inv[:], inv[:], 1.0 / M)

                # Wsum = sum_m inv_m * U_m: tensor_scalar (4x) + adds (2x)
                Wsum = wkp.tile([P, S], fp16, tag="Wsum", name="Wsum")
                tmp2 = wkp.tile([P, S], fp16, tag="tmp2", name="tmp2")
                for m in range(M):
                    um = mt[:, m * S:(m + 1) * S]
                    nc.vector.tensor_scalar(um, um, inv[:, m:m + 1], None,
                                            op0=ALU.mult)
                # first pair-add on the (idle) gpsimd engine: runs in
                # parallel with the DVE's tmp2 add, cutting DVE cadence
                nc.gpsimd.tensor_tensor(Wsum[:], mt[:, 0:S], mt[:, S:2 * S],
                                        op=ALU.add)
                nc.vector.tensor_tensor(tmp2[:], mt[:, 2 * S:3 * S],
                                        mt[:, 3 * S:4 * S], op=ALU.add)
                nc.vector.tensor_tensor(Wsum[:], Wsum[:], tmp2[:], op=ALU.add)

                # transpose Wsum [128, S] -> WT columns via xbar DMA
                nc.sync.dma_start_transpose(
                    WT[:].rearrange("p (i q) -> p i q", i=KCH)[:, :, t * P:(t + 1) * P],
                    Wsum[:])

            # ---- phases F+G interleaved: F(qb) then G for its q-tiles ---
            def g_tile(t):
                ot = wkp.tile([P, D], fp16, tag="ot", name="ot", bufs=2)
                for db in range(D // NB):
                    ps = psp.tile([P, NB], fp32, tag="ps", name="ps")
                    for c in range(DCH):
                        nc.tensor.matmul(
                            ps[:],
                            OT[:, c * SQ + t * P: c * SQ + (t + 1) * P],
                            Wo[:, c * D + db * NB: c * D + (db + 1) * NB],
                            start=(c == 0), stop=(c == DCH - 1))
                    nc.vector.tensor_copy(ot[:, db * NB:(db + 1) * NB], ps[:])
                nc.sync.dma_start(out_d.ap()[t * P:(t + 1) * P, :], ot[:])

            for qb in range(SQ // NB):
                for j in range(DCH):
                    ps = psp.tile([P, NB], fp32, tag="ps", name="ps")
                    for i in range(KCH):
                        nc.tensor.matmul(
                            ps[:],
                            V[:, i * D + j * P: i * D + (j + 1) * P],
                            WT[:, i * SQ + qb * NB: i * SQ + (qb + 1) * NB],
                            start=(i == 0), stop=(i == KCH - 1))
                    nc.vector.tensor_copy(
                        OT[:, j * SQ + qb * NB: j * SQ + (qb + 1) * NB],
                        ps[:])
                for t in range(qb * NB // P, (qb + 1) * NB // P):
                    g_tile(t)
            wk_ctx.__exit__(None, None, None)

    nc.compile()
    return nc


def _get_nc():
    if "nc" not in _CACHE:
        _CACHE["nc"] = build()
    return _CACHE["nc"]


def kernel(x, stride_masks, Wq, bq, Wk, bk, Wv, bv, Wo, bo):
    from concourse import bass_utils

    x = np.asarray(x, dtype=np.float32)
    stride_masks = np.asarray(stride_masks, dtype=np.int32)
    Wq16 = np.ascontiguousarray(np.asarray(Wq, dtype=np.float32).astype(np.float16))
    Wk16 = np.ascontiguousarray(np.asarray(Wk, dtype=np.float32).astype(np.float16))
    Wv16 = np.ascontiguousarray(np.asarray(Wv, dtype=np.float32).astype(np.float16))
    Wo16 = np.ascontiguousarray(np.asarray(Wo, dtype=np.float32).astype(np.float16))
    bq = np.asarray(bq, dtype=np.float32)
    bk = np.asarray(bk, dtype=np.float32)
    bv = np.asarray(bv, dtype=np.float32)
    bo = np.asarray(bo, dtype=np.float32)

    nc = _get_nc()

    # Biases are spec'd zero-fill; the device kernel omits them. bv/bo fold
    # in exactly on the host (softmax rows sum to 1); bq/bk would need a
    # device path, so assert they are zero.
    assert not (np.any(bq) or np.any(bk)), "nonzero q/k bias unsupported"

    # mask bias 30*(mask-1) in {-30, 0}: exp(s+mb) == mask*exp(s) to ~1e-13
    mb_half = [np.ascontiguousarray(
                   (30.0 * (stride_masks[:, h * SQ:(h + 1) * SQ, :] - 1.0)
                    ).astype(np.float16))
               for h in range(2)]
    in_maps = []
    for c in range(N_CORES):
        b, h = c // 2, c % 2
        xT = np.ascontiguousarray(x[b].astype(np.float16).T)
        xTq = np.ascontiguousarray(xT[:, h * SQ:(h + 1) * SQ])
        in_maps.append({
            "xT": xT, "xTq": xTq, "mb": mb_half[h],
            "Wq": Wq16, "Wk": Wk16, "Wv": Wv16, "Wo": Wo16,
        })

    res = bass_utils.run_bass_kernel_spmd(nc, in_maps, core_ids=list(range(N_CORES)))
    _CACHE["last_results"] = res

    out = np.empty((B, S, D), dtype=np.float32)
    for c in range(N_CORES):
        b, h = c // 2, c % 2
        out[b, h * SQ:(h + 1) * SQ, :] = res.results[c]["out"].astype(np.float32)

    if np.any(bv):
        out += (bv @ Wo)[None, None, :]
    if np.any(bo):
        out += bo[None, None, :]
    return out


# revision 27
# speedup vs baseline: 1.0097x; 1.0097x over previous
"""CAMSA multi-mask attention kernel for one TRN2 chip (8 NeuronCores).

Problem: B=4, S=2048, D=1024, M=4 stride masks.
  Q = x@Wq + bq ; K = x@Wk + bk ; V = x@Wv + bv     (biases are zero-fill)
  scores = Q K^T / sqrt(D)                           [B,S,S]
  weights_m = softmax(where(mask_m==0, -1e9, scores))
  out = (mean_m weights_m) @ V @ Wo + bo

Algebra: with s = scores/sqrt(D) (~N(0,1), no row-max needed) and the
host-precomputed mask bias mb_m = 30*(mask_m-1) in {-30, 0}:
  U_m = exp(s + mb_m) = mask_m * exp(s)   (exp(-30) ~ 1e-13 underflows)
  den_m[q] = sum_k U_m[q,k]  (ACT accumulator, free with the exp)
  inv_m = 1/(M*den_m);  Wsum = sum_m inv_m*U_m;  out = Wsum @ V @ Wo

Sharding: core c = (batch b=c//2, query-half h=c%2): 1024 query rows.
K/V are each computed for the OWN half of key rows only (from xTq) and
exchanged within the batch pair by an HBM AllGather over replica groups
[[0,1],[2,3],[4,5],[6,7]] - no duplicated projection work.

All inputs arrive fp16 from the host (x pre-transposed); output fp16.

Device pipeline per core (all matmuls fp16, contraction on partitions):
  A: DMA loads on gpsimd ring: xTq | Wk | Wq | Wv
  B: Kh proj -> [K AllGather] ; Q proj ; Vh proj -> [V AllGather]
     (gathers overlap the next projection on the tensor engine)
  C: per q-tile: scores (PSUM) -> ACT copy*1/sqrt(D) -> s16; per mask:
     DVE tensor_tensor add (2x) sm=s16+mb_m, ACT exp with accum_out
     (den_m free); reciprocal; Wsum via DVE tensor_scalar (4x) + adds.
  E: Wsum -> WsumT via one xbar DMA-transpose per q-tile
  F: out_preT[d,q] = V-tile^T . WsumT
  G: final[q,dout] = out_preT-tile^T . Wo -> DRAM (fp16)
"""

import numpy as np

B, S, D, M = 4, 2048, 1024, 4
SQ = S // 2          # query rows per core
PART = 128
N_CORES = 8

_CACHE = {}


def build(nc_factory=None, S=S, D=D, SQ=SQ, M=M, use_deps=True):
    from concourse import bass, mybir, bacc, tile
    from concourse.tile import add_dep_helper

    fp32 = mybir.dt.float32
    fp16 = mybir.dt.float16
    AF = mybir.ActivationFunctionType
    ALU = mybir.AluOpType

    P = PART
    DCH = D // P         # d-chunks
    KCH = S // P         # key-row chunks
    HKCH = KCH // 2      # key-row chunks per half
    QTILES = SQ // P     # q-tiles per core
    NB = min(512, S, SQ, D)
    GROUPS = [[0, 1], [2, 3], [4, 5], [6, 7]]

    if nc_factory is None:
        nc = bacc.Bacc("TRN2", target_bir_lowering=False, debug=False,
                       num_devices=N_CORES)
    else:
        nc = nc_factory()

    xT_d = nc.dram_tensor("xT", [D, S], fp16, kind="ExternalInput")
    xTq_d = nc.dram_tensor("xTq", [D, SQ], fp16, kind="ExternalInput")
    mk_d = nc.dram_tensor("mb", [M, SQ, S], fp16, kind="ExternalInput")
    wq_d = nc.dram_tensor("Wq", [D, D], fp16, kind="ExternalInput")
    wk_d = nc.dram_tensor("Wk", [D, D], fp16, kind="ExternalInput")
    wv_d = nc.dram_tensor("Wv", [D, D], fp16, kind="ExternalInput")
    wo_d = nc.dram_tensor("Wo", [D, D], fp16, kind="ExternalInput")
    out_d = nc.dram_tensor("out", [SQ, D], fp16, kind="ExternalOutput")

    with tile.TileContext(nc) as tc:
        with tc.tile_pool(name="persist", bufs=1) as pp, \
             tc.tile_pool(name="psum", bufs=8, space="PSUM") as psp, \
             tc.tile_pool(name="dram", bufs=1, space="DRAM") as dr:

            QT = pp.tile([P, DCH * SQ], fp16)    # [p, j*SQ+q] = Q[q, j*128+p]
            KT = pp.tile([P, DCH * S], fp16, tag="KT")  # [p,j*S+k] = K[k,j*128+p]
            V = pp.tile([P, KCH * D], fp16)      # [p, i*D+d]  = V[i*128+p, d]

            def wload(dst, src_d):
                return nc.gpsimd.dma_start(
                    dst[:].rearrange("p (c d) -> p c d", c=DCH),
                    src_d.ap().rearrange("(c p) d -> p c d", p=P))

            def proj(dst, w_sb, src_sb, ncols):
                # dst[p, j*ncols+r] = sum_dx W[dx, j*128+p] * src[dx, r]
                for j in range(DCH):
                    for qb in range(ncols // NB):
                        ps = psp.tile([P, NB], fp32, tag="ps", name="ps")
                        for c in range(DCH):
                            nc.tensor.matmul(
                                ps[:],
                                w_sb[:, c * D + j * P: c * D + (j + 1) * P],
                                src_sb[:, c * ncols + qb * NB: c * ncols + (qb + 1) * NB],
                                start=(c == 0), stop=(c == DCH - 1))
                        nc.vector.tensor_copy(
                            dst[:, j * ncols + qb * NB: j * ncols + (qb + 1) * NB],
                            ps[:])

            # ---- phase A/B: load + half projections + pair gathers ------
            with tc.tile_pool(name="stage", bufs=1) as sw:
                xTq = sw.tile([P, DCH * SQ], fp16, name="xTq")
                Wq = sw.tile([P, DCH * D], fp16, name="Wq")
                Wk = sw.tile([P, DCH * D], fp16, name="Wk")
                Wv = sw.tile([P, DCH * D], fp16, name="Wv")
                xT = sw.tile([P, DCH * S], fp16, name="xT")
                # xTq on the sync ring, parallel with the gp load chain
                nc.sync.dma_start(
                    xTq[:].rearrange("p (c r) -> p c r", c=DCH),
                    xTq_d.ap().rearrange("(c p) r -> p c r", p=P))
                d_wq = wload(Wq, wq_d)
                d_xt = nc.gpsimd.dma_start(
                    xT[:].rearrange("p (c r) -> p c r", c=DCH),
                    xT_d.ap().rearrange("(c p) r -> p c r", p=P))
                d_wk = wload(Wk, wk_d)
                d_wv = wload(Wv, wv_d)
                if use_deps:
                    add_dep_helper(d_xt.ins, d_wq.ins, sync=False, reason="dma order")
                    add_dep_helper(d_wk.ins, d_xt.ins, sync=False, reason="dma order")
                    add_dep_helper(d_wv.ins, d_wk.ins, sync=False, reason="dma order")

                proj(QT, Wq, xTq, SQ)
                proj(KT, Wk, xT, S)

                # V projection (full key range): lhsT = xT chunk, rhs = Wv
                for il in range(KCH):
                    for db in range(D // NB):
                        ps = psp.tile([P, NB], fp32, tag="ps", name="ps")
                        for c in range(DCH):
                            nc.tensor.matmul(
                                ps[:],
                                xT[:, c * S + il * P: c * S + (il + 1) * P],
                                Wv[:, c * D + db * NB: c * D + (db + 1) * NB],
                                start=(c == 0), stop=(c == DCH - 1))
                        nc.vector.tensor_copy(
                            V[:, il * D + db * NB: il * D + (db + 1) * NB],
                            ps[:])

            # ---- work pools for phases C/E/F/G --------------------------
            wk_ctx = tc.tile_pool(name="work", bufs=2)
            wkp = wk_ctx.__enter__()
            WT = wkp.tile([P, KCH * SQ], fp16, name="WT", tag="WT", bufs=1)
            #    [p, i*SQ+q] = Wsum[q, i*128+p]
            OT = wkp.tile([P, DCH * SQ], fp16, name="OT", tag="OT", bufs=1)
            #    [p, j*SQ+q] = out_pre[q, j*128+p]
            # mask-bias tiles [p, m*S+k] = 30*(mask[m, t*128+p, k]-1) stream
            # on the sync ring, two tiles ahead of consumption, so no
            # collective ever head-blocks them.
            mts = [None] * QTILES

            def issue_mt(t):
                mts[t] = wkp.tile([P, M * S], fp16, tag="mt", name="mt",
                                  bufs=3)
                nc.sync.dma_start(
                    mts[t][:].rearrange("p (m k) -> p m k", m=M),
                    mk_d.ap()[:, t * P:(t + 1) * P, :].transpose([1, 0, 2]))

            issue_mt(0)
            issue_mt(1)

            # ---- phase C/E: scores -> U_m -> Wsum -> WsumT --------------
            inv_scale = 1.0 / float(np.sqrt(np.float32(D)))
            for t in range(QTILES):
                if t + 2 < QTILES:
                    issue_mt(t + 2)
                mt = mts[t]

                s16 = wkp.tile([P, S], fp16, tag="s16", name="s16")
                for kb in range(S // NB):
                    ps = psp.tile([P, NB], fp32, tag="ps", name="ps")
                    for c in range(DCH):
                        nc.tensor.matmul(
                            ps[:],
                            QT[:, c * SQ + t * P: c * SQ + (t + 1) * P],
                            KT[:, c * S + kb * NB: c * S + (kb + 1) * NB],
                            start=(c == 0), stop=(c == DCH - 1))
                    nc.scalar.activation(
                        s16[:, kb * NB:(kb + 1) * NB], ps[:],
                        AF.Copy, scale=inv_scale)

                if t == QTILES - 1:
                    # Wo reuses KT's slot (KT dead after last scores).
                    Wo = pp.tile([P, DCH * D], fp16, name="Wo", tag="KT")
                    wload(Wo, wo_d)

                den = wkp.tile([P, M], fp32, tag="den", name="den")
                # sm_m = s + mb_m on DVE (2x); U_m = exp(sm_m) on ACT with
                # the row-sum accumulated for free.
                for m in range(M):
                    um = mt[:, m * S:(m + 1) * S]
                    nc.vector.tensor_tensor(um, um, s16[:], op=ALU.add)
                    nc.scalar.activation(um, um, AF.Exp,
                                         accum_out=den[:, m:m + 1])
                inv = wkp.tile([P, M], fp32, tag="inv", name="inv")
                nc.vector.reciprocal(inv[:], den[:])
                nc.vector.tensor_scalar_mul(inv[:], inv[:], 1.0 / M)

                # Wsum = sum_m inv_m * U_m: tensor_scalar (4x) + adds (2x)
                Wsum = wkp.tile([P, S], fp16, tag="Wsum", name="Wsum")
                tmp2 = wkp.tile([P, S], fp16, tag="tmp2", name="tmp2")
                for m in range(M):
                    um = mt[:, m * S:(m + 1) * S]
                    nc.vector.tensor_scalar(um, um, inv[:, m:m + 1], None,
                                            op0=ALU.mult)
                nc.vector.tensor_tensor(Wsum[:], mt[:, 0:S], mt[:, S:2 * S],
                                        op=ALU.add)
                nc.vector.tensor_tensor(tmp2[:], mt[:, 2 * S:3 * S],
                                        mt[:, 3 * S:4 * S], op=ALU.add)
                nc.vector.tensor_tensor(Wsum[:], Wsum[:], tmp2[:], op=ALU.add)

                # transpose Wsum [128, S] -> WT columns via xbar DMA
                nc.sync.dma_start_transpose(
                    WT[:].rearrange("p (i q) -> p i q", i=KCH)[:, :, t * P:(t + 1) * P],
                    Wsum[:])

            # ---- phases F+G interleaved: F(qb) then G for its q-tiles ---
            def g_tile(t):
                ot = wkp.tile([P, D], fp16, tag="ot", name="ot", bufs=2)
                for db in range(D // NB):
                    ps = psp.tile([P, NB], fp32, tag="ps", name="ps")
                    for c in range(DCH):
                        nc.tensor.matmul(
                            ps[:],
                            OT[:, c * SQ + t * P: c * SQ + (t + 1) * P],
                            Wo[:, c * D + db * NB: c * D + (db + 1) * NB],
                            start=(c == 0), stop=(c == DCH - 1))
                    nc.vector.tensor_copy(ot[:, db * NB:(db + 1) * NB], ps[:])
                nc.sync.dma_start(out_d.ap()[t * P:(t + 1) * P, :], ot[:])

            for qb in range(SQ // NB):
                for j in range(DCH):
                    ps = psp.tile([P, NB], fp32, tag="ps", name="ps")
                    for i in range(KCH):
                        nc.tensor.matmul(
                            ps[:],
                            V[:, i * D + j * P: i * D + (j + 1) * P],
                            WT[:, i * SQ + qb * NB: i * SQ + (qb + 1) * NB],
                            start=(i == 0), stop=(i == KCH - 1))
                    nc.vector.tensor_copy(
                        OT[:, j * SQ + qb * NB: j * SQ + (qb + 1) * NB],
                        ps[:])
                for t in range(qb * NB // P, (qb + 1) * NB // P):
                    g_tile(t)
            wk_ctx.__exit__(None, None, None)

    nc.compile()
    return nc


def _get_nc():
    if "nc" not in _CACHE:
        _CACHE["nc"] = build()
    return _CACHE["nc"]


def kernel(x, stride_masks, Wq, bq, Wk, bk, Wv, bv, Wo, bo):
    from concourse import bass_utils

    x = np.asarray(x, dtype=np.float32)
    stride_masks = np.asarray(stride_masks, dtype=np.int32)
    Wq16 = np.ascontiguousarray(np.asarray(Wq, dtype=np.float32).astype(np.float16))
    Wk16 = np.ascontiguousarray(np.asarray(Wk, dtype=np.float32).astype(np.float16))
    Wv16 = np.ascontiguousarray(np.asarray(Wv, dtype=np.float32).astype(np.float16))
    Wo16 = np.ascontiguousarray(np.asarray(Wo, dtype=np.float32).astype(np.float16))
    bq = np.asarray(bq, dtype=np.float32)
    bk = np.asarray(bk, dtype=np.float32)
    bv = np.asarray(bv, dtype=np.float32)
    bo = np.asarray(bo, dtype=np.float32)

    nc = _get_nc()

    # Biases are spec'd zero-fill; the device kernel omits them. bv/bo fold
    # in exactly on the host (softmax rows sum to 1); bq/bk would need a
    # device path, so assert they are zero.
    assert not (np.any(bq) or np.any(bk)), "nonzero q/k bias unsupported"

    # mask bias 30*(mask-1) in {-30, 0}: exp(s+mb) == mask*exp(s) to ~1e-13
    mb_half = [np.ascontiguousarray(
                   (30.0 * (stride_masks[:, h * SQ:(h + 1) * SQ, :] - 1.0)
                    ).astype(np.float16))
               for h in range(2)]
    in_maps = []
    for c in range(N_CORES):
        b, h = c // 2, c % 2
        xT = np.ascontiguousarray(x[b].astype(np.float16).T)
        xTq = np.ascontiguousarray(xT[:, h * SQ:(h + 1) * SQ])
        in_maps.append({
            "xT": xT, "xTq": xTq, "mb": mb_half[h],
            "Wq": Wq16, "Wk": Wk16, "Wv": Wv16, "Wo": Wo16,
        })

    res = bass_utils.run_bass_kernel_spmd(nc, in_maps, core_ids=list(range(N_CORES)))
    _CACHE["last_results"] = res

    out = np.empty((B, S, D), dtype=np.float32)
    for c in range(N_CORES):
        b, h = c // 2, c % 2
        out[b, h * SQ:(h + 1) * SQ, :] = res.results[c]["out"].astype(np.float32)

    if np.any(bv):
        out += (bv @ Wo)[None, None, :]
    if np.any(bo):
        out += bo[None, None, :]
    return out
